# revision 56
# baseline (speedup 1.0000x reference)
"""KAN-SE (squeeze-excite with 2-layer KAN MLP) Trainium2 kernel.

Full-input contract: kernel(**inputs) takes the complete (32, 512, 64, 64)
batch plus KAN weights, shards the batch across 8 NeuronCores (4 samples
per core, data-parallel, weights replicated), and returns the full output.

The rel-err gate is 2e-2 (fp32 pipeline measured 4e-7), so precision is
traded for bandwidth/throughput (verified ~4e-4 l2 end to end): x/y move
over HBM as fp16 (host casts both ways), KAN weights/features are fp16 on
the PE, sums/activations stay f32.

Pipeline (v2, rebuilt from per-packet DMA traces of v1: the 16 DMA
engines cap at ~424 GB/s aggregate, so the whole game is keeping them
byte-streaming end to end; typical HW time 101-104us vs v1's 108-115us):
  - all 16 x-tile loads ride the Sync-engine HWDGE queue (the Sync engine
    runs nothing else, so its in-order doorbells just stream; one queue
    saturates all 16 engines for reads).  Stores alternate BOTH HWDGE
    queues (a single queue only sustains ~340 GB/s for writes), with
    doorbells from ScalarE/Sync rung in gate-completion order so a late
    gate never blocks another transfer.
  - consts ride the Act queue in parallel with the x stream: gtab (tiny,
    needed first) immediately; the 1.77 MiB merged weight tensor's
    doorbell is rung from ScalarE's idle gap after the sample-0 sums so
    it never delays the pair's last x tile (which gates the whole store
    stream).  Layer-2 weights are zero-padded to 128 partitions inside
    that one merged tensor; the layer-2 matmul contracts over 128
    partitions with memset-zeroed feature rows on top.
  - row-sums are split per the SUM_ENG map: DVE fp16 half-add (~1.2us)
    feeding either a ScalarE Copy+accum on the 2048 half (~2.0us, "H")
    or a DVE accum ("V").  The pair's final tile uses a monolithic
    ScalarE Copy+accum ("S", no DVE dep); s2/s3's final tiles use two
    preemptible ScalarE halves + DVE combine ("P") — a 3.7us monolith
    there gets statically ordered ahead of the previous batch's tiny l2
    gate sigmoids on in-order ScalarE and stalls the store stream ~3us.
    GpSimd is unusable for real work: its software tensor_scalar takes
    ~59us and wedges concurrent DVE ops (measured).
  - KAN for samples {0,1} runs pair-batched; samples 2 and 3 run alone
    so a late-landing s3 tile can never stall s2's gates (robustness
    under cross-core HBM slowdowns, which vary run to run).  Layer-2
    runs per out-group: each 9-matmul chain feeds sigmoid -> DVE gate
    scale -> store doorbell, so stores dribble og-by-og.  s2's group-0..2
    spline block is hoisted ahead of the pair's og2/og3 scales (whose
    store slots come ~8us later) so s2's gates meet the store stream.
  - timeline on a clean run: preamble ~8us; loads stream 8-50us at ~420
    GB/s with consts folded in; pair gates are ready ~44us so stores
    take over seamlessly; s2/s3 gates stay ahead of their store slots;
    stream ends ~97us + ~9us fixed teardown barrier.

Per-core HBM traffic: 16 MiB in + 16 MiB out (fp16), read-once/write-once.
"""

import numpy as np

# ---- problem constants (hardcoded per contract; do not read spec/reference) ----
B, C, H, W = 32, 512, 64, 64
HIDDEN = 64            # max(16, 512 // 8)
KB = 8                 # GRID_SIZE + SPLINE_ORDER = 5 + 3
NCORES = 8
NS = B // NCORES       # samples per core = 4
NG = C // 128          # channel groups of 128 = 4
HWPIX = H * W          # 4096
NF = KB + 1            # features per channel: silu + 8 spline bases

# row-sum mode per tile index t = n*4+g: "H" = DVE half-add (fp16 2x,
# ~1.2us) then ScalarE Copy+accum on the 2048 half (~1.9us) — splits each
# sum across both engines; "S" = monolithic ScalarE Copy+accum (~3.7us,
# no DVE dependency — used for each KAN batch's final tile so the
# b-spline chain on DVE never waits behind it); "V" = DVE half-add + DVE
# accum (~3.5us, keeps ScalarE free)
SUM_ENG = "VHVH" "VHHS" "HHHP" "HHHP"
# gate-multiply engine per tile index: V = DVE (fp16 2x, ~1.3us),
# S = ScalarE Copy scale (~3.9us).  GpSimd is NOT usable: its software
# tensor_scalar takes ~59us and wedges concurrent DVE ops (measured).
SCALE_ENG = "VVVV" "VVVV" "VVVV" "VVVV"
# DMA trigger queue per tile index: S = Sync-engine HWDGE queue,
# A = ScalarE (Activation) HWDGE queue (see docstring).
LQUEUE = "SSSS" "SSSS" "SSSS" "SSSS"
SQUEUE = "SASA" "ASAS" "SASA" "ASAS"


def _grid_cols(grid_row: np.ndarray, xscale: float, nsg: int):
    """Packed per-group-replicated grid constant columns for the batched
    Cox-de-Boor recurrence, evaluated on inputs x' = x * xscale.

    offsets maps:
      'ge'   -> start of g_i * xscale,        width nsg*12
      (k,'a')-> start of -g_i / (k h),        width nsg*(11-k)
      (k,'c')-> start of  g_{i+k+1} / (k h),  width nsg*(11-k)
      'rs'   -> start of 1/(k h xscale), k=1..3
    """
    g = np.asarray(grid_row, np.float64)
    assert g.shape == (12,)
    h = g[1] - g[0]
    segs, offsets = [], {}
    pos = 0

    def add(key, vals):
        nonlocal pos
        offsets[key] = pos
        segs.append(vals.astype(np.float32))
        pos += vals.size

    add('ge', np.tile(g * xscale, nsg))
    for k in (1, 2, 3):
        w = 11 - k
        add((k, 'a'), np.tile(-g[:w] / (k * h), nsg))
        add((k, 'c'), np.tile(g[k + 1:12] / (k * h), nsg))
    add('rs', np.array([1.0 / (k * h * xscale) for k in (1, 2, 3)]))
    return np.concatenate(segs), offsets


# c128ext column layout (fp16, 128 partitions):
#   [0,     256)   w1t   (layer-1 base weights, 1/HW folded in)
#   [256,   2304)  sw1   (layer-1 spline*scaler)
#   [2304,  2816)  w2t   (layer-2 base weights, zero rows 64-127)
#   [2816,  6912)  sw2   (layer-2 spline*scaler, zero rows 64-127)
W1C = NG * HIDDEN + NG * KB * HIDDEN       # 2304
W2TO = W1C                                  # w2t start
SW2O = W1C + C                              # sw2 start
CEXT = W1C + C + KB * C                     # 6912 total cols


def _host_prep(inputs):
    """Rearrange weights into the SBUF layouts the device program uses."""
    f32, f16 = np.float32, np.float16
    base_w1 = np.asarray(inputs["base_w1"], f32)      # (64, 512)
    spline_w1 = np.asarray(inputs["spline_w1"], f32)  # (64, 512, 8)
    scaler1 = np.asarray(inputs["scaler1"], f32)      # (64, 512)
    base_w2 = np.asarray(inputs["base_w2"], f32)      # (512, 64)
    spline_w2 = np.asarray(inputs["spline_w2"], f32)  # (512, 64, 8)
    scaler2 = np.asarray(inputs["scaler2"], f32)      # (512, 64)

    # layer-1 silu feature arrives as sum*sigmoid(sum/HW) = HW*silu(mean),
    # so fold 1/HW into the base weights.
    # w1t[p, g*64+o] = base_w1[o, 128g+p] / HWPIX
    w1t = (base_w1 / HWPIX).reshape(HIDDEN, NG, 128)
    w1t = w1t.transpose(2, 1, 0).reshape(128, NG * HIDDEN)
    # sw1[p, (g*8+k)*64+o] = (spline_w1*scaler1)[o, 128g+p, k]
    sw1 = (spline_w1 * scaler1[:, :, None]).reshape(HIDDEN, NG, 128, KB)
    sw1 = sw1.transpose(2, 1, 3, 0).reshape(128, NG * KB * HIDDEN)
    # layer-2 weights zero-padded to 128 partitions (rows 64-127 zero, so
    # the 128-partition contraction ignores the undefined feature rows)
    w2t = np.zeros((128, C), f32)
    w2t[:HIDDEN] = base_w2.T
    sw2 = np.zeros((128, KB * C), f32)
    sw2[:HIDDEN] = (spline_w2 * scaler2[:, :, None]).transpose(1, 2, 0).reshape(
        HIDDEN, KB * C)

    # packed grid-constant table: layer1 (on raw sums, xscale=HW) then
    # layer2 (xscale=1); single copy each, broadcast on-device over
    # samples and groups with stride-0 APs
    c1, off1 = _grid_cols(np.asarray(inputs["grid1"], f32)[0], float(HWPIX), 1)
    c2, off2 = _grid_cols(np.asarray(inputs["grid2"], f32)[0], 1.0, 1)
    off2 = {k: v + c1.size for k, v in off2.items()}
    gtab = np.concatenate([c1, c2])
    gtab_full = np.ascontiguousarray(np.tile(gtab[None, :], (128, 1)))

    cext = np.concatenate([w1t, sw1, w2t, sw2], axis=1).astype(f16)
    assert cext.shape == (128, CEXT)
    tensors = {
        "c128": np.ascontiguousarray(cext),
        "gtab": gtab_full,
    }
    return tensors, off1, off2, gtab.size


def _emit_bsplines(nc, mybir, pool, gtab_sb, off, sT3, out_j, P, S, G, g0=0):
    """Cubic B-spline bases for S*G per-partition scalars at once.

    sT3:   AP [P, S, G] of the (pre-scaled) inputs.
    out_j: AP [P, S, G, 8] (may be strided, fp16) for the final bases.
    g0:    first group index (selects the replicated grid-constant cols).
    Grid constants broadcast over S (stride-0); x broadcasts over the basis
    index, so each Cox-de-Boor level is one DVE op over ~S*G*11 elems.
    """
    f32 = mybir.dt.float32
    Alu = mybir.AluOpType

    def rep(key, w):
        o = off[key]
        return gtab_sb[:P, o:o + w].rearrange(
            "p i -> p () () i").broadcast_to([P, S, G, w])

    ge = pool.tile([128, S, G, 12], f32, tag=f"ge{P}{S}{G}", bufs=2)
    xb = sT3.rearrange("p s g -> p s g ()")
    nc.vector.tensor_tensor(
        ge[:P], rep('ge', 12), xb.broadcast_to([P, S, G, 12]), Alu.is_le)
    bprev = pool.tile([128, S, G, 11], f32, tag=f"b0{P}{S}{G}", bufs=2)
    nc.vector.tensor_tensor(
        bprev[:P], ge[:P, :, :, 0:11], ge[:P, :, :, 1:12], Alu.subtract)
    # xr[p, k, s, g] = x * 1/(k h xscale)
    xr = pool.tile([128, 3, S, G], f32, tag=f"xr{P}{S}{G}", bufs=2)
    o = off['rs']
    rs_ap = gtab_sb[:P, o:o + 3].rearrange("p k -> p k () ()")
    nc.vector.tensor_tensor(
        xr[:P], rs_ap.broadcast_to([P, 3, S, G]),
        sT3.rearrange("p s g -> p () s g").broadcast_to([P, 3, S, G]), Alu.mult)
    for k in (1, 2, 3):
        w = 11 - k
        xk = xr[:P, k - 1].rearrange("p s g -> p s g ()").broadcast_to([P, S, G, w])
        a_t = pool.tile([128, S, G, 10], f32, tag=f"bsA{P}{S}{G}", bufs=2)
        c_t = pool.tile([128, S, G, 10], f32, tag=f"bsC{P}{S}{G}", bufs=2)
        # A = (x - g_i)/(k h) = xr + (-g_i/(k h));  C = g_{i+k+1}/(k h) - xr
        nc.vector.tensor_tensor(a_t[:P, :, :, :w], rep((k, 'a'), w), xk, Alu.add)
        nc.vector.tensor_tensor(c_t[:P, :, :, :w], rep((k, 'c'), w), xk, Alu.subtract)
        if k < 3:
            bnext = pool.tile([128, S, G, 10], f32, tag=f"bn{P}{S}{G}", bufs=2)
            outp = bnext[:P, :, :, :w]
        else:
            outp = out_j
        nc.vector.tensor_tensor(
            c_t[:P, :, :, :w], c_t[:P, :, :, :w], bprev[:P, :, :, 1:w + 1], Alu.mult)
        nc.vector.tensor_tensor(outp, a_t[:P, :, :, :w], bprev[:P, :, :, 0:w], Alu.mult)
        nc.vector.tensor_tensor(outp, outp, c_t[:P, :, :, :w], Alu.add)
        if k < 3:
            bprev = bnext


def _build_nc(off1, off2, gtab_cols):
    import concourse.bacc as bacc
    import concourse.bass as bass  # noqa: F401
    import concourse.mybir as mybir
    from concourse.tile import TileContext

    f32 = mybir.dt.float32
    f16 = mybir.dt.float16
    Alu = mybir.AluOpType
    Act = mybir.ActivationFunctionType

    # Bacc (not plain Bass): its compile() runs move_matmul_waits_to_ldweights
    # + generate_event_semaphores, which split multi-waits down to the 1-wait-
    # per-instruction TRN2 ISA limit that walrus enforces.
    nc = bacc.Bacc("TRN2", target_bir_lowering=False)
    x_d = nc.declare_dram_parameter("x", [NS, C, H, W], f16, isOutput=False)
    c128_d = nc.declare_dram_parameter("c128", [128, CEXT], f16, isOutput=False)
    gtab_d = nc.declare_dram_parameter("gtab", [128, gtab_cols], f32, isOutput=False)
    y_d = nc.declare_dram_parameter("y", [NS, C, H, W], f16, isOutput=True)

    with TileContext(nc) as tc:
        with (
            tc.tile_pool(name="consts", bufs=1) as cpool,
            tc.tile_pool(name="xdata", bufs=NS * NG) as xpool,
            tc.tile_pool(name="small", bufs=NS) as spool,
            tc.tile_pool(name="bspl", bufs=1) as bpool,
            tc.tile_pool(name="psum", bufs=2, space="PSUM") as ppool,
        ):
            # ---- consts on the Act queue first (they stream in parallel
            # with the x loads and warm the Act DGE for the stores), then
            # all 16 x-load doorbells on the Sync queue in sample order ----
            xts = {}
            for n in range(NS):
                for g in range(NG):
                    xt = xpool.tile([128, HWPIX], f16, tag="xt")
                    xts[(n, g)] = xt

            c128_sb = cpool.tile([128, CEXT], f16)
            gtab_sb = cpool.tile([128, gtab_cols], f32)
            # gtab is tiny and needed first (b-splines ~t+28); its 128 small
            # descriptors cost the idle Act DGE ~3us but steal no bandwidth.
            # c128's doorbell is rung later (see below) so the first ~27us
            # of HBM bandwidth all goes to the pair-1 x tiles.
            nc.scalar.dma_start(gtab_sb[:], gtab_d[:, :])

            def load(n, g):
                src = x_d[n, 128 * g:128 * (g + 1)].rearrange("p h w -> p (h w)")
                eng = nc.sync if LQUEUE[NG * n + g] == "S" else nc.scalar
                eng.dma_start(xts[(n, g)][:], src)

            for n in range(NS):
                for g in range(NG):
                    load(n, g)
            w1t_sb = c128_sb[:, 0:NG * HIDDEN]
            sw1_sb = c128_sb[:, NG * HIDDEN:W1C]
            w2t_sb = c128_sb[:, W2TO:W2TO + C]
            sw2_sb = c128_sb[:, SW2O:CEXT]
            touch = cpool.tile([128, 8], f32)
            # ScalarE: preload BOTH act tables before its first accum; the
            # source must carry no DMA wait (consts land ~25us in), so read
            # a gpsimd-zeroed scratch column.
            nc.gpsimd.memset(touch[:, 7:8], 0)
            nc.scalar.activation(touch[:, 5:6], touch[:, 7:8], Act.Sigmoid)
            nc.scalar.activation(touch[:, 6:7], touch[:, 7:8], Act.Copy)
            # TensorE pre-touch: the c128 DMA-completion wait lands on this
            # throwaway matmul (LDWEIGHTS has a single wait slot).
            pt_ps = ppool.tile([1, 4], f32, tag="pt", bufs=1)
            nc.tensor.matmul(pt_ps[0:1, 0:1], c128_sb[:, 0:1], c128_sb[:, 0:1],
                             start=True, stop=True)

            # samples 0,1 share one sums tile (their KAN runs pair-batched);
            # samples 2 and 3 run alone so a late-landing s3 tile can never
            # stall s2's gates (robustness under cross-core HBM slowdowns)
            sT01 = spool.tile([128, 2 * NG], f32, tag="sT01", bufs=1)
            sT2 = spool.tile([128, NG], f32, tag="sT", bufs=2)
            sT3 = spool.tile([128, NG], f32, tag="sT", bufs=2)
            scols = {}
            for n in range(NS):
                for g in range(NG):
                    if n < 2:
                        scols[(n, g)] = sT01[:, NG * n + g:NG * n + g + 1]
                    elif n == 2:
                        scols[(n, g)] = sT2[:, g:g + 1]
                    else:
                        scols[(n, g)] = sT3[:, g:g + 1]

            def emit_sums(tiles):
                for n, g in tiles:
                    t = NG * n + g
                    xt = xts[(n, g)]
                    scol = scols[(n, g)]
                    eng = SUM_ENG[t]
                    if eng == "S":
                        nc.scalar.activation(xt[:], xt[:], Act.Copy,
                                             accum_out=scol)
                        continue
                    if eng == "P":
                        # two preemptible ScalarE halves + DVE combine: a
                        # monolithic 3.7us sum here gets statically ordered
                        # ahead of the PREVIOUS batch's tiny l2 gate
                        # sigmoids on in-order ScalarE and stalls the store
                        # stream ~3us (measured); 2us halves let the
                        # scheduler slot the sigmoids between them
                        pc = spool.tile([128, 2], f32, tag="pc", bufs=4)
                        for c in range(2):
                            h_ap = xt[:, HWPIX // 2 * c:HWPIX // 2 * (c + 1)]
                            nc.scalar.activation(h_ap, h_ap, Act.Copy,
                                                 accum_out=pc[:, c:c + 1])
                        nc.vector.reduce_sum(scol, pc[:],
                                             axis=mybir.AxisListType.X)
                        continue
                    # half-add tree: plain fp16 TT runs ~2x the accum
                    # path, so fold 4096->2048 on DVE first, then accum
                    # the half on ScalarE ("H") or DVE ("V")
                    half = bpool.tile([128, HWPIX // 2], f16,
                                      tag="half", bufs=4)
                    nc.vector.tensor_tensor(
                        half[:], xt[:, 0:HWPIX // 2], xt[:, HWPIX // 2:],
                        Alu.add)
                    if eng == "H":
                        nc.scalar.activation(half[:], half[:], Act.Copy,
                                             accum_out=scol)
                    else:
                        nc.vector.tensor_scalar(
                            out=half[:], in0=half[:], scalar1=1.0,
                            scalar2=None, op0=Alu.mult, op1=Alu.add,
                            accum_out=scol)

            # ---- KAN instances: pair (0,1), then singles 2, 3.
            # mid() emits the batch's final-tile sum (S-mode, no DVE dep)
            # after the group-0..2 b-splines so only the group-3 emit and 9
            # matmuls chain behind it; late() emits the NEXT batch's early
            # sums between out-groups 1 and 2 so the DVE tile-sum work
            # overlaps this batch's l2/scale phase without delaying it. ----
            def emit_g012(samples, sT):
                """Standalone emit of a KAN batch's group-0..2 b-splines
                (pure DVE work depending only on the batch's first NG*S-1
                tile sums) — lets a later batch's spline block be hoisted
                ahead of this batch's low-deadline gate scales."""
                S = len(samples)
                bfta = spool.tile([128, NF * S * 3], f16, tag=f"bfta{S}")
                sT3 = sT[:, 0:S * NG].rearrange("p (s g) -> p s g", g=NG)
                out_j = bfta.rearrange("p (j s g) -> p s g j", s=S, g=3)
                _emit_bsplines(nc, mybir, bpool, gtab_sb, off1,
                               sT3[:, :, 0:3], out_j[:, :, :, 1:NF],
                               128, S, 3, g0=0)
                return bfta

            def kan(samples, sT, mid=None, late=None, bft_pre=None):
                S = len(samples)
                # layer-1 features live in TWO tiles so dependency tracking
                # separates them: bfta (groups 0-2, col (j*S+s)*3+g) is
                # ready before the batch's final tile lands; bftb (group 3,
                # col j*S+s) chains behind it.  27 of the 36 layer-1
                # matmuls therefore start ~2us earlier (with one tile the
                # whole chain waited on the group-3 spline emit, measured).
                # j=0 is the silu feature sum*sigmoid(sum/HW) (the 1/HW
                # lives in w1t); its sigmoid is split the same way.
                bfta = bft_pre if bft_pre is not None else emit_g012(samples, sT)
                bfta4 = bfta.rearrange("p (j s g) -> p j s g", s=S, g=3)
                bftb = spool.tile([128, NF * S], f16, tag=f"bftb{S}")
                bftb4 = bftb.rearrange("p (j s g) -> p j s g", s=S, g=1)
                sT3 = sT[:, 0:S * NG].rearrange("p (s g) -> p s g", g=NG)
                sig1a = spool.tile([128, S * 3], f32, tag=f"sig1a{S}")
                s1a3 = sig1a.rearrange("p (s g) -> p s g", g=3)
                nc.scalar.activation(s1a3, sT3[:, :, 0:3], Act.Sigmoid,
                                     scale=1.0 / HWPIX)
                nc.vector.tensor_tensor(bfta4[:, 0], s1a3, sT3[:, :, 0:3],
                                        Alu.mult)
                if mid is not None:
                    mid()
                sig1b = spool.tile([128, S], f32, tag=f"sig1b{S}")
                s1b3 = sig1b.rearrange("p (s g) -> p s g", g=1)
                nc.scalar.activation(s1b3, sT3[:, :, 3:4], Act.Sigmoid,
                                     scale=1.0 / HWPIX)
                nc.vector.tensor_tensor(bftb4[:, 0], s1b3, sT3[:, :, 3:4],
                                        Alu.mult)
                # group 3 alone: only this emit and 9 matmuls depend on the
                # batch's final tile
                _emit_bsplines(nc, mybir, bpool, gtab_sb, off1,
                               sT3[:, :, 3:4],
                               bftb.rearrange("p (j s g) -> p s g j",
                                              s=S, g=1)[:, :, :, 1:NF],
                               128, S, 1, g0=3)

                # layer-1 matmuls: one accumulation chain over (g, j)
                ps1 = ppool.tile([HIDDEN, S], f32, tag=f"ps1{S}", bufs=(1 if S == 2 else 2))
                nmm = NG * NF
                i = 0
                for g in range(NG):
                    for j in range(NF):
                        if j == 0:
                            lhsT = w1t_sb[:, HIDDEN * g:HIDDEN * (g + 1)]
                        else:
                            col = HIDDEN * (KB * g + (j - 1))
                            lhsT = sw1_sb[:, col:col + HIDDEN]
                        rhs = bfta4[:, j, :, g] if g < 3 else bftb4[:, j, :, 0]
                        nc.tensor.matmul(
                            ps1[:], lhsT, rhs,
                            start=(i == 0), stop=(i == nmm - 1))
                        i += 1

                # inter-layer: t1 = silu(ps1); layer-2 features on 128
                # partitions (rows 64-127 zeroed to match the padded weights)
                sg = spool.tile([HIDDEN, 2 * S], f32, tag=f"sg2{S}")
                t1 = spool.tile([HIDDEN, S], f32, tag=f"t1{S}")
                nc.scalar.activation(sg[:, 0:S], ps1[:], Act.Sigmoid)
                nc.vector.tensor_tensor(t1[:], sg[:, 0:S], ps1[:], Alu.mult)
                bf2 = spool.tile([128, NF * S], f16, tag=f"bf2{S}")
                nc.gpsimd.memset(bf2[HIDDEN:128, :], 0)
                bf24 = bf2.rearrange("p (j s g) -> p j s g", s=S, g=1)
                nc.scalar.activation(sg[:, S:2 * S], t1[:], Act.Sigmoid)
                nc.vector.tensor_tensor(bf24[:HIDDEN, 0, :, 0], sg[:, S:2 * S],
                                        t1[:], Alu.mult)
                _emit_bsplines(nc, mybir, bpool, gtab_sb, off2,
                               t1.rearrange("p (s g) -> p s g", g=1),
                               bf2.rearrange("p (j s g) -> p s g j", s=S, g=1)
                               [:HIDDEN, :, :, 1:NF],
                               HIDDEN, S, 1)

                # layer-2 per out-group: 9-matmul chain -> sigmoid -> gate
                # multiply -> store doorbell (stores dribble og-by-og)
                for og in range(NG):
                    if og == 2 and late is not None:
                        late()
                    ps2 = ppool.tile([128, S], f32, tag=f"ps2{S}", bufs=2)
                    for j in range(NF):
                        if j == 0:
                            lhsT = w2t_sb[:, 128 * og:128 * (og + 1)]
                        else:
                            col = C * (j - 1) + 128 * og
                            lhsT = sw2_sb[:, col:col + 128]
                        nc.tensor.matmul(
                            ps2[:], lhsT, bf24[:, j, :, 0],
                            start=(j == 0), stop=(j == NF - 1))
                    gate = spool.tile([128, S], f32, tag=f"gate{S}", bufs=4)
                    nc.scalar.activation(gate[:], ps2[:], Act.Sigmoid)
                    for si, n in enumerate(samples):
                        t = NG * n + og
                        xt = xts[(n, og)]
                        gcol = gate[:, si:si + 1]
                        if SCALE_ENG[t] == "V":
                            nc.vector.tensor_scalar(
                                out=xt[:], in0=xt[:], scalar1=gcol,
                                scalar2=None, op0=Alu.mult)
                        else:
                            nc.scalar.activation(xt[:], xt[:], Act.Copy,
                                                 scale=gcol)
                        dst = y_d[n, 128 * og:128 * (og + 1)].rearrange(
                            "p h w -> p (h w)")
                        seng = nc.sync if SQUEUE[t] == "S" else nc.scalar
                        seng.dma_start(dst, xt[:])

            emit_sums([(0, 0), (0, 1), (0, 2), (0, 3)])
            # c128 doorbell here: ScalarE rings it in its idle gap between
            # the s0 and s1 accums; the transfer (1.77 MiB) lands by ~t+26,
            # ahead of the first layer-1 matmul, without delaying the pair's
            # last x tile (which gates the whole store stream).
            # 2 descriptors per partition keeps each under the efficient 8KB.
            nc.scalar.dma_start(
                c128_sb.rearrange("p (a b) -> p a b", a=2),
                c128_d[:, :].rearrange("p (a b) -> p a b", a=2))
            emit_sums([(1, 0), (1, 1), (1, 2)])
            # DVE pre-touch of gtab right before the first b-spline block:
            # the DMA-completion wait lands here, not on the b-spline ops.
            nc.vector.tensor_copy(touch[:, 0:1], gtab_sb[:, 0:1])
            # s2's group-0..2 spline block is hoisted into the pair's
            # og-loop (emitted before the pair's og2/og3 scales, whose
            # store slots come ~8us later than s2og0's): its sums + splines
            # then race ahead so s2's gates meet the store stream.
            holder = {}

            def late_pair():
                emit_sums([(2, 0), (2, 1), (2, 2)])
                holder["bft2"] = emit_g012([2], sT2)

            # s3's group-0..2 splines are hoisted into s2's og-loop the
            # same way, ahead of s2's og2/og3 scales (store slots ~8us
            # later), so s3's 27 early layer-1 matmuls and gates also meet
            # the store stream.
            def late_s2():
                emit_sums([(3, 0), (3, 1), (3, 2)])
                holder["bft3"] = emit_g012([3], sT3)

            kan([0, 1], sT01,
                mid=lambda: emit_sums([(1, 3)]),
                late=late_pair)
            kan([2], sT2,
                mid=lambda: emit_sums([(2, 3)]),
                late=late_s2,
                bft_pre=holder["bft2"])
            kan([3], sT3,
                mid=lambda: emit_sums([(3, 3)]),
                bft_pre=holder["bft3"])
    nc.compile()
    return nc


def _run(inputs, trace=False):
    from concourse.bass_utils import run_bass_kernel_spmd

    x = np.asarray(inputs["x"])
    assert x.shape == (B, C, H, W), x.shape
    x16 = np.ascontiguousarray(x.astype(np.float16))
    tensors, off1, off2, gtab_cols = _host_prep(inputs)
    nc = _build_nc(off1, off2, gtab_cols)
    in_maps = []
    for c in range(NCORES):
        m = {"x": np.ascontiguousarray(x16[NS * c:NS * (c + 1)])}
        m.update(tensors)
        in_maps.append(m)
    res = run_bass_kernel_spmd(
        nc, in_maps, core_ids=list(range(NCORES)), trace=trace
    )
    out = np.concatenate([res.results[c]["y"] for c in range(NCORES)], axis=0)
    return out.astype(np.float32), res


def kernel(**inputs) -> np.ndarray:
    return _run(inputs)[0]


# revision 58
# speedup vs baseline: 1.1706x; 1.1706x over previous
"""KAN-SE (squeeze-excite with 2-layer KAN MLP) Trainium2 kernel.

Full-input contract: kernel(**inputs) takes the complete (32, 512, 64, 64)
batch plus KAN weights, shards the batch across 8 NeuronCores (4 samples
per core, data-parallel, weights replicated), and returns the full output.

The rel-err gate is 2e-2 (fp32 pipeline measured 4e-7), so precision is
traded for bandwidth/throughput (verified ~4e-4 l2 end to end): x/y move
over HBM as fp16 (host casts both ways), KAN weights/features are fp16 on
the PE, sums/activations stay f32.

Pipeline (v2, rebuilt from per-packet DMA traces of v1: the 16 DMA
engines cap at ~424 GB/s aggregate, so the whole game is keeping them
byte-streaming end to end; typical HW time 101-104us vs v1's 108-115us):
  - all 16 x-tile loads ride the Sync-engine HWDGE queue (the Sync engine
    runs nothing else, so its in-order doorbells just stream; one queue
    saturates all 16 engines for reads).  Stores alternate BOTH HWDGE
    queues (a single queue only sustains ~340 GB/s for writes), with
    doorbells from ScalarE/Sync rung in gate-completion order so a late
    gate never blocks another transfer.
  - consts ride the Act queue in parallel with the x stream: gtab (tiny,
    needed first) immediately; the 1.77 MiB merged weight tensor's
    doorbell is rung from ScalarE's idle gap after the sample-0 sums so
    it never delays the pair's last x tile (which gates the whole store
    stream).  Layer-2 weights are zero-padded to 128 partitions inside
    that one merged tensor; the layer-2 matmul contracts over 128
    partitions with memset-zeroed feature rows on top.
  - row-sums are split per the SUM_ENG map: DVE fp16 half-add (~1.2us)
    feeding either a ScalarE Copy+accum on the 2048 half (~2.0us, "H")
    or a DVE accum ("V").  The pair's final tile uses a monolithic
    ScalarE Copy+accum ("S", no DVE dep); s2/s3's final tiles use two
    preemptible ScalarE halves + DVE combine ("P") — a 3.7us monolith
    there gets statically ordered ahead of the previous batch's tiny l2
    gate sigmoids on in-order ScalarE and stalls the store stream ~3us.
    GpSimd is unusable for real work: its software tensor_scalar takes
    ~59us and wedges concurrent DVE ops (measured).
  - KAN for samples {0,1} runs pair-batched; samples 2 and 3 run alone
    so a late-landing s3 tile can never stall s2's gates (robustness
    under cross-core HBM slowdowns, which vary run to run).  Layer-2
    runs per out-group: each 9-matmul chain feeds sigmoid -> DVE gate
    scale -> store doorbell, so stores dribble og-by-og.  s2's group-0..2
    spline block is hoisted ahead of the pair's og2/og3 scales (whose
    store slots come ~8us later) so s2's gates meet the store stream.
  - timeline on a clean run: preamble ~8us; loads stream 8-50us at ~420
    GB/s with consts folded in; pair gates are ready ~44us so stores
    take over seamlessly; s2/s3 gates stay ahead of their store slots;
    stream ends ~97us + ~9us fixed teardown barrier.

Per-core HBM traffic: 16 MiB in + 16 MiB out (fp16), read-once/write-once.
"""

import numpy as np

# ---- problem constants (hardcoded per contract; do not read spec/reference) ----
B, C, H, W = 32, 512, 64, 64
HIDDEN = 64            # max(16, 512 // 8)
KB = 8                 # GRID_SIZE + SPLINE_ORDER = 5 + 3
NCORES = 8
NS = B // NCORES       # samples per core = 4
NG = C // 128          # channel groups of 128 = 4
HWPIX = H * W          # 4096
NF = KB + 1            # features per channel: silu + 8 spline bases

# row-sum mode per tile index t = n*4+g: "H" = DVE half-add (fp16 2x,
# ~1.2us) then ScalarE Copy+accum on the 2048 half (~1.9us) — splits each
# sum across both engines; "S" = monolithic ScalarE Copy+accum (~3.7us,
# no DVE dependency — used for each KAN batch's final tile so the
# b-spline chain on DVE never waits behind it); "V" = DVE half-add + DVE
# accum (~3.5us, keeps ScalarE free)
SUM_ENG = "VHVH" "VHHS" "HHHP" "HHHP"
# gate-multiply engine per tile index: V = DVE (fp16 2x, ~1.3us),
# S = ScalarE Copy scale (~3.9us).  GpSimd is NOT usable: its software
# tensor_scalar takes ~59us and wedges concurrent DVE ops (measured).
SCALE_ENG = "VVVV" "VVVV" "VVVV" "VVVV"
# DMA trigger queue per tile index: S = Sync-engine HWDGE queue,
# A = ScalarE (Activation) HWDGE queue (see docstring).
LQUEUE = "SSSS" "SSSS" "SSSS" "SSSS"
SQUEUE = "SASA" "ASAS" "SASA" "ASAS"


def _grid_cols(grid_row: np.ndarray, xscale: float, nsg: int):
    """Packed per-group-replicated grid constant columns for the batched
    Cox-de-Boor recurrence, evaluated on inputs x' = x * xscale.

    offsets maps:
      'ge'   -> start of g_i * xscale,        width nsg*12
      (k,'a')-> start of -g_i / (k h),        width nsg*(11-k)
      (k,'c')-> start of  g_{i+k+1} / (k h),  width nsg*(11-k)
      'rs'   -> start of 1/(k h xscale), k=1..3
    """
    g = np.asarray(grid_row, np.float64)
    assert g.shape == (12,)
    h = g[1] - g[0]
    segs, offsets = [], {}
    pos = 0

    def add(key, vals):
        nonlocal pos
        offsets[key] = pos
        segs.append(vals.astype(np.float32))
        pos += vals.size

    add('ge', np.tile(g * xscale, nsg))
    for k in (1, 2, 3):
        w = 11 - k
        add((k, 'a'), np.tile(-g[:w] / (k * h), nsg))
        add((k, 'c'), np.tile(g[k + 1:12] / (k * h), nsg))
    add('rs', np.array([1.0 / (k * h * xscale) for k in (1, 2, 3)]))
    return np.concatenate(segs), offsets


# c128ext column layout (fp16, 128 partitions):
#   [0,     256)   w1t   (layer-1 base weights, 1/HW folded in)
#   [256,   2304)  sw1   (layer-1 spline*scaler)
#   [2304,  2816)  w2t   (layer-2 base weights, zero rows 64-127)
#   [2816,  6912)  sw2   (layer-2 spline*scaler, zero rows 64-127)
W1C = NG * HIDDEN + NG * KB * HIDDEN       # 2304
W2TO = W1C                                  # w2t start
SW2O = W1C + C                              # sw2 start
CEXT = W1C + C + KB * C                     # 6912 total cols


def _host_prep(inputs):
    """Rearrange weights into the SBUF layouts the device program uses."""
    f32, f16 = np.float32, np.float16
    base_w1 = np.asarray(inputs["base_w1"], f32)      # (64, 512)
    spline_w1 = np.asarray(inputs["spline_w1"], f32)  # (64, 512, 8)
    scaler1 = np.asarray(inputs["scaler1"], f32)      # (64, 512)
    base_w2 = np.asarray(inputs["base_w2"], f32)      # (512, 64)
    spline_w2 = np.asarray(inputs["spline_w2"], f32)  # (512, 64, 8)
    scaler2 = np.asarray(inputs["scaler2"], f32)      # (512, 64)

    # layer-1 silu feature arrives as sum*sigmoid(sum/HW) = HW*silu(mean),
    # so fold 1/HW into the base weights.
    # w1t[p, g*64+o] = base_w1[o, 128g+p] / HWPIX
    w1t = (base_w1 / HWPIX).reshape(HIDDEN, NG, 128)
    w1t = w1t.transpose(2, 1, 0).reshape(128, NG * HIDDEN)
    # sw1[p, (g*8+k)*64+o] = (spline_w1*scaler1)[o, 128g+p, k]
    sw1 = (spline_w1 * scaler1[:, :, None]).reshape(HIDDEN, NG, 128, KB)
    sw1 = sw1.transpose(2, 1, 3, 0).reshape(128, NG * KB * HIDDEN)
    # layer-2 weights zero-padded to 128 partitions (rows 64-127 zero, so
    # the 128-partition contraction ignores the undefined feature rows)
    w2t = np.zeros((128, C), f32)
    w2t[:HIDDEN] = base_w2.T
    sw2 = np.zeros((128, KB * C), f32)
    sw2[:HIDDEN] = (spline_w2 * scaler2[:, :, None]).transpose(1, 2, 0).reshape(
        HIDDEN, KB * C)

    # packed grid-constant table: layer1 (on raw sums, xscale=HW) then
    # layer2 (xscale=1); single copy each, broadcast on-device over
    # samples and groups with stride-0 APs
    c1, off1 = _grid_cols(np.asarray(inputs["grid1"], f32)[0], float(HWPIX), 1)
    c2, off2 = _grid_cols(np.asarray(inputs["grid2"], f32)[0], 1.0, 1)
    off2 = {k: v + c1.size for k, v in off2.items()}
    gtab = np.concatenate([c1, c2])
    gtab_full = np.ascontiguousarray(np.tile(gtab[None, :], (128, 1)))

    cext = np.concatenate([w1t, sw1, w2t, sw2], axis=1).astype(f16)
    assert cext.shape == (128, CEXT)
    tensors = {
        "c128": np.ascontiguousarray(cext),
        "gtab": gtab_full,
    }
    return tensors, off1, off2, gtab.size


def _emit_bsplines(nc, mybir, pool, gtab_sb, off, sT3, out_j, P, S, G, g0=0):
    """Cubic B-spline bases for S*G per-partition scalars at once.

    sT3:   AP [P, S, G] of the (pre-scaled) inputs.
    out_j: AP [P, S, G, 8] (may be strided, fp16) for the final bases.
    g0:    first group index (selects the replicated grid-constant cols).
    Grid constants broadcast over S (stride-0); x broadcasts over the basis
    index, so each Cox-de-Boor level is one DVE op over ~S*G*11 elems.
    """
    f32 = mybir.dt.float32
    Alu = mybir.AluOpType

    def rep(key, w):
        o = off[key]
        return gtab_sb[:P, o:o + w].rearrange(
            "p i -> p () () i").broadcast_to([P, S, G, w])

    ge = pool.tile([128, S, G, 12], f32, tag=f"ge{P}{S}{G}", bufs=2)
    xb = sT3.rearrange("p s g -> p s g ()")
    nc.vector.tensor_tensor(
        ge[:P], rep('ge', 12), xb.broadcast_to([P, S, G, 12]), Alu.is_le)
    bprev = pool.tile([128, S, G, 11], f32, tag=f"b0{P}{S}{G}", bufs=2)
    nc.vector.tensor_tensor(
        bprev[:P], ge[:P, :, :, 0:11], ge[:P, :, :, 1:12], Alu.subtract)
    # xr[p, k, s, g] = x * 1/(k h xscale)
    xr = pool.tile([128, 3, S, G], f32, tag=f"xr{P}{S}{G}", bufs=2)
    o = off['rs']
    rs_ap = gtab_sb[:P, o:o + 3].rearrange("p k -> p k () ()")
    nc.vector.tensor_tensor(
        xr[:P], rs_ap.broadcast_to([P, 3, S, G]),
        sT3.rearrange("p s g -> p () s g").broadcast_to([P, 3, S, G]), Alu.mult)
    for k in (1, 2, 3):
        w = 11 - k
        xk = xr[:P, k - 1].rearrange("p s g -> p s g ()").broadcast_to([P, S, G, w])
        a_t = pool.tile([128, S, G, 10], f32, tag=f"bsA{P}{S}{G}", bufs=2)
        c_t = pool.tile([128, S, G, 10], f32, tag=f"bsC{P}{S}{G}", bufs=2)
        # A = (x - g_i)/(k h) = xr + (-g_i/(k h));  C = g_{i+k+1}/(k h) - xr
        nc.vector.tensor_tensor(a_t[:P, :, :, :w], rep((k, 'a'), w), xk, Alu.add)
        nc.vector.tensor_tensor(c_t[:P, :, :, :w], rep((k, 'c'), w), xk, Alu.subtract)
        if k < 3:
            bnext = pool.tile([128, S, G, 10], f32, tag=f"bn{P}{S}{G}", bufs=2)
            outp = bnext[:P, :, :, :w]
        else:
            outp = out_j
        nc.vector.tensor_tensor(
            c_t[:P, :, :, :w], c_t[:P, :, :, :w], bprev[:P, :, :, 1:w + 1], Alu.mult)
        nc.vector.tensor_tensor(outp, a_t[:P, :, :, :w], bprev[:P, :, :, 0:w], Alu.mult)
        nc.vector.tensor_tensor(outp, outp, c_t[:P, :, :, :w], Alu.add)
        if k < 3:
            bprev = bnext


def _build_nc(off1, off2, gtab_cols):
    import concourse.bacc as bacc
    import concourse.bass as bass  # noqa: F401
    import concourse.mybir as mybir
    from concourse.tile import TileContext

    f32 = mybir.dt.float32
    f16 = mybir.dt.float16
    Alu = mybir.AluOpType
    Act = mybir.ActivationFunctionType

    # Bacc (not plain Bass): its compile() runs move_matmul_waits_to_ldweights
    # + generate_event_semaphores, which split multi-waits down to the 1-wait-
    # per-instruction TRN2 ISA limit that walrus enforces.
    nc = bacc.Bacc("TRN2", target_bir_lowering=False)
    x_d = nc.declare_dram_parameter("x", [NS, C, H, W], f16, isOutput=False)
    c128_d = nc.declare_dram_parameter("c128", [128, CEXT], f16, isOutput=False)
    gtab_d = nc.declare_dram_parameter("gtab", [128, gtab_cols], f32, isOutput=False)
    y_d = nc.declare_dram_parameter("y", [NS, C, H, W], f16, isOutput=True)

    with TileContext(nc) as tc:
        with (
            tc.tile_pool(name="consts", bufs=1) as cpool,
            tc.tile_pool(name="xdata", bufs=NS * NG) as xpool,
            tc.tile_pool(name="small", bufs=NS) as spool,
            tc.tile_pool(name="bspl", bufs=1) as bpool,
            tc.tile_pool(name="psum", bufs=2, space="PSUM") as ppool,
        ):
            # ---- consts on the Act queue first (they stream in parallel
            # with the x loads and warm the Act DGE for the stores), then
            # all 16 x-load doorbells on the Sync queue in sample order ----
            xts = {}
            for n in range(NS):
                for g in range(NG):
                    xt = xpool.tile([128, HWPIX], f16, tag="xt")
                    xts[(n, g)] = xt

            c128_sb = cpool.tile([128, CEXT], f16)
            gtab_sb = cpool.tile([128, gtab_cols], f32)
            # gtab is tiny and needed first (b-splines ~t+28); its 128 small
            # descriptors cost the idle Act DGE ~3us but steal no bandwidth.
            # c128's doorbell is rung later (see below) so the first ~27us
            # of HBM bandwidth all goes to the pair-1 x tiles.
            nc.scalar.dma_start(gtab_sb[:], gtab_d[:, :])

            def load(n, g):
                src = x_d[n, 128 * g:128 * (g + 1)].rearrange("p h w -> p (h w)")
                eng = nc.sync if LQUEUE[NG * n + g] == "S" else nc.scalar
                eng.dma_start(xts[(n, g)][:], src)

            for n in range(NS):
                for g in range(NG):
                    load(n, g)
            w1t_sb = c128_sb[:, 0:NG * HIDDEN]
            sw1_sb = c128_sb[:, NG * HIDDEN:W1C]
            w2t_sb = c128_sb[:, W2TO:W2TO + C]
            sw2_sb = c128_sb[:, SW2O:CEXT]
            touch = cpool.tile([128, 8], f32)
            # ScalarE: preload BOTH act tables before its first accum; the
            # source must carry no DMA wait (consts land ~25us in), so read
            # a gpsimd-zeroed scratch column.
            nc.gpsimd.memset(touch[:, 7:8], 0)
            nc.scalar.activation(touch[:, 5:6], touch[:, 7:8], Act.Sigmoid)
            nc.scalar.activation(touch[:, 6:7], touch[:, 7:8], Act.Copy)
            # TensorE pre-touch: the c128 DMA-completion wait lands on this
            # throwaway matmul (LDWEIGHTS has a single wait slot).
            pt_ps = ppool.tile([1, 4], f32, tag="pt", bufs=1)
            nc.tensor.matmul(pt_ps[0:1, 0:1], c128_sb[:, 0:1], c128_sb[:, 0:1],
                             start=True, stop=True)

            # samples 0,1 share one sums tile (their KAN runs pair-batched);
            # samples 2 and 3 run alone so a late-landing s3 tile can never
            # stall s2's gates (robustness under cross-core HBM slowdowns)
            sT01 = spool.tile([128, 2 * NG], f32, tag="sT01", bufs=1)
            sT2 = spool.tile([128, NG], f32, tag="sT", bufs=2)
            sT3 = spool.tile([128, NG], f32, tag="sT", bufs=2)
            scols = {}
            for n in range(NS):
                for g in range(NG):
                    if n < 2:
                        scols[(n, g)] = sT01[:, NG * n + g:NG * n + g + 1]
                    elif n == 2:
                        scols[(n, g)] = sT2[:, g:g + 1]
                    else:
                        scols[(n, g)] = sT3[:, g:g + 1]

            def emit_sums(tiles):
                for n, g in tiles:
                    t = NG * n + g
                    xt = xts[(n, g)]
                    scol = scols[(n, g)]
                    eng = SUM_ENG[t]
                    if eng == "S":
                        nc.scalar.activation(xt[:], xt[:], Act.Copy,
                                             accum_out=scol)
                        continue
                    if eng == "P":
                        # two preemptible ScalarE halves + DVE combine: a
                        # monolithic 3.7us sum here gets statically ordered
                        # ahead of the PREVIOUS batch's tiny l2 gate
                        # sigmoids on in-order ScalarE and stalls the store
                        # stream ~3us (measured); 2us halves let the
                        # scheduler slot the sigmoids between them
                        pc = spool.tile([128, 2], f32, tag="pc", bufs=4)
                        for c in range(2):
                            h_ap = xt[:, HWPIX // 2 * c:HWPIX // 2 * (c + 1)]
                            nc.scalar.activation(h_ap, h_ap, Act.Copy,
                                                 accum_out=pc[:, c:c + 1])
                        nc.vector.reduce_sum(scol, pc[:],
                                             axis=mybir.AxisListType.X)
                        continue
                    # half-add tree: plain fp16 TT runs ~2x the accum
                    # path, so fold 4096->2048 on DVE first, then accum
                    # the half on ScalarE ("H") or DVE ("V")
                    half = bpool.tile([128, HWPIX // 2], f16,
                                      tag="half", bufs=4)
                    nc.vector.tensor_tensor(
                        half[:], xt[:, 0:HWPIX // 2], xt[:, HWPIX // 2:],
                        Alu.add)
                    if eng == "H":
                        nc.scalar.activation(half[:], half[:], Act.Copy,
                                             accum_out=scol)
                    else:
                        nc.vector.tensor_scalar(
                            out=half[:], in0=half[:], scalar1=1.0,
                            scalar2=None, op0=Alu.mult, op1=Alu.add,
                            accum_out=scol)

            # ---- KAN instances: pair (0,1), then singles 2, 3.
            # mid() emits the batch's final-tile sum (S-mode, no DVE dep)
            # after the group-0..2 b-splines so only the group-3 emit and 9
            # matmuls chain behind it; late() emits the NEXT batch's early
            # sums between out-groups 1 and 2 so the DVE tile-sum work
            # overlaps this batch's l2/scale phase without delaying it. ----
            def emit_g012(samples, sT):
                """Standalone emit of a KAN batch's group-0..2 b-splines
                (pure DVE work depending only on the batch's first NG*S-1
                tile sums) — lets a later batch's spline block be hoisted
                ahead of this batch's low-deadline gate scales."""
                S = len(samples)
                bfta = spool.tile([128, NF * S * 3], f16, tag=f"bfta{S}")
                sT3 = sT[:, 0:S * NG].rearrange("p (s g) -> p s g", g=NG)
                out_j = bfta.rearrange("p (j s g) -> p s g j", s=S, g=3)
                _emit_bsplines(nc, mybir, bpool, gtab_sb, off1,
                               sT3[:, :, 0:3], out_j[:, :, :, 1:NF],
                               128, S, 3, g0=0)
                return bfta

            def kan(samples, sT, mid=None, late=None, bft_pre=None):
                S = len(samples)
                # layer-1 features live in TWO tiles so dependency tracking
                # separates them: bfta (groups 0-2, col (j*S+s)*3+g) is
                # ready before the batch's final tile lands; bftb (group 3,
                # col j*S+s) chains behind it.  27 of the 36 layer-1
                # matmuls therefore start ~2us earlier (with one tile the
                # whole chain waited on the group-3 spline emit, measured).
                # j=0 is the silu feature sum*sigmoid(sum/HW) (the 1/HW
                # lives in w1t); its sigmoid is split the same way.
                bfta = bft_pre if bft_pre is not None else emit_g012(samples, sT)
                bfta4 = bfta.rearrange("p (j s g) -> p j s g", s=S, g=3)
                bftb = spool.tile([128, NF * S], f16, tag=f"bftb{S}")
                bftb4 = bftb.rearrange("p (j s g) -> p j s g", s=S, g=1)
                sT3 = sT[:, 0:S * NG].rearrange("p (s g) -> p s g", g=NG)
                sig1a = spool.tile([128, S * 3], f32, tag=f"sig1a{S}")
                s1a3 = sig1a.rearrange("p (s g) -> p s g", g=3)
                nc.scalar.activation(s1a3, sT3[:, :, 0:3], Act.Sigmoid,
                                     scale=1.0 / HWPIX)
                nc.vector.tensor_tensor(bfta4[:, 0], s1a3, sT3[:, :, 0:3],
                                        Alu.mult)
                if mid is not None:
                    mid()
                sig1b = spool.tile([128, S], f32, tag=f"sig1b{S}")
                s1b3 = sig1b.rearrange("p (s g) -> p s g", g=1)
                nc.scalar.activation(s1b3, sT3[:, :, 3:4], Act.Sigmoid,
                                     scale=1.0 / HWPIX)
                nc.vector.tensor_tensor(bftb4[:, 0], s1b3, sT3[:, :, 3:4],
                                        Alu.mult)
                # group 3 alone: only this emit and 9 matmuls depend on the
                # batch's final tile
                _emit_bsplines(nc, mybir, bpool, gtab_sb, off1,
                               sT3[:, :, 3:4],
                               bftb.rearrange("p (j s g) -> p s g j",
                                              s=S, g=1)[:, :, :, 1:NF],
                               128, S, 1, g0=3)

                # layer-1 matmuls: one accumulation chain over (g, j)
                ps1 = ppool.tile([HIDDEN, S], f32, tag=f"ps1{S}", bufs=(1 if S == 2 else 2))
                nmm = NG * NF
                i = 0
                for g in range(NG):
                    for j in range(NF):
                        if j == 0:
                            lhsT = w1t_sb[:, HIDDEN * g:HIDDEN * (g + 1)]
                        else:
                            col = HIDDEN * (KB * g + (j - 1))
                            lhsT = sw1_sb[:, col:col + HIDDEN]
                        rhs = bfta4[:, j, :, g] if g < 3 else bftb4[:, j, :, 0]
                        nc.tensor.matmul(
                            ps1[:], lhsT, rhs,
                            start=(i == 0), stop=(i == nmm - 1))
                        i += 1

                # inter-layer: t1 = silu(ps1); layer-2 features on 128
                # partitions (rows 64-127 zeroed to match the padded weights)
                sg = spool.tile([HIDDEN, 2 * S], f32, tag=f"sg2{S}")
                t1 = spool.tile([HIDDEN, S], f32, tag=f"t1{S}")
                nc.scalar.activation(sg[:, 0:S], ps1[:], Act.Sigmoid)
                nc.vector.tensor_tensor(t1[:], sg[:, 0:S], ps1[:], Alu.mult)
                bf2 = spool.tile([128, NF * S], f16, tag=f"bf2{S}")
                nc.gpsimd.memset(bf2[HIDDEN:128, :], 0)
                bf24 = bf2.rearrange("p (j s g) -> p j s g", s=S, g=1)
                nc.scalar.activation(sg[:, S:2 * S], t1[:], Act.Sigmoid)
                nc.vector.tensor_tensor(bf24[:HIDDEN, 0, :, 0], sg[:, S:2 * S],
                                        t1[:], Alu.mult)
                _emit_bsplines(nc, mybir, bpool, gtab_sb, off2,
                               t1.rearrange("p (s g) -> p s g", g=1),
                               bf2.rearrange("p (j s g) -> p s g j", s=S, g=1)
                               [:HIDDEN, :, :, 1:NF],
                               HIDDEN, S, 1)

                # layer-2 per out-group: 9-matmul chain -> sigmoid -> gate
                # multiply -> store doorbell (stores dribble og-by-og)
                for og in range(NG):
                    if og == 2 and late is not None:
                        late()
                    ps2 = ppool.tile([128, S], f32, tag=f"ps2{S}", bufs=2)
                    for j in range(NF):
                        if j == 0:
                            lhsT = w2t_sb[:, 128 * og:128 * (og + 1)]
                        else:
                            col = C * (j - 1) + 128 * og
                            lhsT = sw2_sb[:, col:col + 128]
                        nc.tensor.matmul(
                            ps2[:], lhsT, bf24[:, j, :, 0],
                            start=(j == 0), stop=(j == NF - 1))
                    gate = spool.tile([128, S], f32, tag=f"gate{S}", bufs=4)
                    nc.scalar.activation(gate[:], ps2[:], Act.Sigmoid)
                    for si, n in enumerate(samples):
                        t = NG * n + og
                        xt = xts[(n, og)]
                        gcol = gate[:, si:si + 1]
                        if SCALE_ENG[t] == "V":
                            nc.vector.tensor_scalar(
                                out=xt[:], in0=xt[:], scalar1=gcol,
                                scalar2=None, op0=Alu.mult)
                        else:
                            nc.scalar.activation(xt[:], xt[:], Act.Copy,
                                                 scale=gcol)
                        dst = y_d[n, 128 * og:128 * (og + 1)].rearrange(
                            "p h w -> p (h w)")
                        seng = nc.sync if SQUEUE[t] == "S" else nc.scalar
                        seng.dma_start(dst, xt[:])

            emit_sums([(0, 0), (0, 1), (0, 2), (0, 3)])
            # c128 doorbell here: ScalarE rings it in its idle gap between
            # the s0 and s1 accums; the transfer (1.77 MiB) lands by ~t+26,
            # ahead of the first layer-1 matmul, without delaying the pair's
            # last x tile (which gates the whole store stream).
            # 2 descriptors per partition keeps each under the efficient 8KB.
            nc.scalar.dma_start(
                c128_sb.rearrange("p (a b) -> p a b", a=2),
                c128_d[:, :].rearrange("p (a b) -> p a b", a=2))
            emit_sums([(1, 0), (1, 1), (1, 2)])
            # DVE pre-touch of gtab right before the first b-spline block:
            # the DMA-completion wait lands here, not on the b-spline ops.
            nc.vector.tensor_copy(touch[:, 0:1], gtab_sb[:, 0:1])
            # s2's group-0..2 spline block is hoisted into the pair's
            # og-loop (emitted before the pair's og2/og3 scales, whose
            # store slots come ~8us later than s2og0's): its sums + splines
            # then race ahead so s2's gates meet the store stream.
            holder = {}

            def late_pair():
                emit_sums([(2, 0), (2, 1), (2, 2)])
                holder["bft2"] = emit_g012([2], sT2)

            # s3's group-0..2 splines are hoisted into s2's og-loop the
            # same way, ahead of s2's og2/og3 scales (store slots ~8us
            # later), so s3's 27 early layer-1 matmuls and gates also meet
            # the store stream.
            def late_s2():
                emit_sums([(3, 0), (3, 1), (3, 2)])
                holder["bft3"] = emit_g012([3], sT3)

            kan([0, 1], sT01,
                mid=lambda: emit_sums([(1, 3)]),
                late=late_pair)
            kan([2], sT2,
                mid=lambda: emit_sums([(2, 3)]),
                late=late_s2,
                bft_pre=holder["bft2"])
            kan([3], sT3,
                mid=lambda: emit_sums([(3, 3)]),
                bft_pre=holder["bft3"])
    nc.compile()
    return nc


def _run(inputs, trace=False):
    from concourse.bass_utils import run_bass_kernel_spmd

    x = np.asarray(inputs["x"])
    assert x.shape == (B, C, H, W), x.shape
    x16 = np.ascontiguousarray(x.astype(np.float16))
    tensors, off1, off2, gtab_cols = _host_prep(inputs)
    nc = _build_nc(off1, off2, gtab_cols)
    in_maps = []
    for c in range(NCORES):
        m = {"x": np.ascontiguousarray(x16[NS * c:NS * (c + 1)])}
        m.update(tensors)
        in_maps.append(m)
    res = run_bass_kernel_spmd(
        nc, in_maps, core_ids=list(range(NCORES)), trace=trace
    )
    out = np.concatenate([res.results[c]["y"] for c in range(NCORES)], axis=0)
    return out.astype(np.float32), res


def kernel(**inputs) -> np.ndarray:
    return _run(inputs)[0]
